# revision 16
# baseline (speedup 1.0000x reference)
"""Embedding-lookup MF model kernel for Trainium2 (8 NeuronCores).

reference math (B = 16384, D = 64):
    u   = user_table[x[:, 0]]          # [B, D]
    v   = item_table[x[:, 1]]          # [B, D]
    out = sigmoid(sum(u * v, -1))      # [B]

Strategy: data-parallel across the batch, with HOST-side index prep that
lets each core fetch all its rows with FIVE InstDMAGatherAnt instructions
instead of 32 per-partition indirect DMAs (SWDGE fixed cost is ~1 us per
instruction, so instruction count dominates).

dma_gather takes int16 indices (< 32768) into a row window whose base is a
compile-time AP offset, and writes gathered row i of the instruction to
dst[i % 128, i // 128, :]. To make every index fit in 16 bits:

  - batch rows are sorted globally by user id and dealt to the 8 cores in
    2048-row quantile spans: each core's user ids then span ~12.5k values
    (< 32768 with huge margin), so ONE u-gather per core from a per-core
    user window (the window is sliced host-side and uploaded per core).
  - within a core, its 2048 rows are sorted by item id; the 4 runs of 512
    consecutive sorted item ids each span ~25k values (< 32768), giving
    FOUR v-gathers per core from per-run item windows.

Per-core uploaded table: [5 * 32768, 64] f32 = 40 MB (u window + 4 v
windows). Index tile: [128, 256] int16 (idx i of an instruction lives at
partition i%16, col i//16, replicated 8x down the partition dim for the 8
Q7 cores). Host un-permutes the [128, 16] result tiles at the end.

The v-gather of run r overlaps the DVE mul+reduce and ACT sigmoid of run
r-1; the output store is per-run so only run 3's chain sits on the tail.
"""

import os

# A previously crashed process can leave the NeuronCores wedged
# (NRT_EXEC_UNIT_UNRECOVERABLE on the next run); requesting a core reset at
# runtime init is harmless otherwise and self-heals that state.
os.environ.setdefault("NEURON_RT_RESET_CORES", "1")

import numpy as np

import concourse.mybir as mybir
import concourse.tile as tile
from concourse import bacc, library_config
from concourse.bass_utils import run_bass_kernel_spmd

N_CORES = 8
P = 128
D = 64
B = 16384
BPC = B // N_CORES  # 2048 batch rows per core
NBLK = BPC // P  # 16 column blocks of 128 batch rows
WIN = 32768  # dma_gather int16 index window (rows)
VRUNS = 4
VRUN = BPC // VRUNS  # 512 positions per v-run
VBLK = VRUN // P  # 4 blocks per v-run
UCOLS = BPC // 16  # 128 idx columns for the u gather
VCOLS = VRUN // 16  # 32 idx columns per v run

_programs: dict = {}


def _build():
    """Single-core program, run SPMD on 8 cores."""
    nc = bacc.Bacc(
        "TRN2",
        target_bir_lowering=False,
        debug=False,
        detect_race_conditions=False,
        num_swdge_queues=4,
    )
    idx = nc.dram_tensor(
        "idx", [P, UCOLS + VRUNS * VCOLS], mybir.dt.int16, kind="ExternalInput"
    )
    tbl = nc.dram_tensor(
        "tbl", [(1 + VRUNS) * WIN, D], mybir.dt.float32, kind="ExternalInput"
    )
    out = nc.dram_tensor("out", [P, NBLK], mybir.dt.float32, kind="ExternalOutput")

    with tile.TileContext(nc) as tc:
        with (
            tc.tile_pool(name="io", bufs=1) as io_pool,
            tc.tile_pool(name="prod", bufs=2) as prod_pool,
        ):
            t_idx = io_pool.tile([P, UCOLS + VRUNS * VCOLS], mybir.dt.int16)
            # idx load goes through SWDGE *before* the library reload: its
            # completion semaphore then fires while the reload still runs,
            # instead of queueing behind the reload's IRAM traffic
            nc.gpsimd.dma_start(out=t_idx[:], in_=idx[:])
            # Q7 IRAM reload for the gather library; blocks the Q7 cluster
            # ~6.5 us, overlapping the idx transfer above
            nc.gpsimd.load_library(library_config.mlp)
            tu = io_pool.tile([P, BPC // P * D], mybir.dt.float32)
            tv = io_pool.tile([P, BPC // P * D], mybir.dt.float32)
            t_res = io_pool.tile([P, NBLK], mybir.dt.float32)
            t_bias = io_pool.tile([P, 1], mybir.dt.float32)
            nc.vector.memset(t_bias[:], 0.0)

            tu3 = tu[:].rearrange("p (n d) -> p n d", d=D)
            tv3 = tv[:].rearrange("p (n d) -> p n d", d=D)

            # Q7 descriptor generation runs at ~9 ns/descriptor per core
            # pair, and SWDGE queue r dispatches to its own core pair — so
            # split the work into a (u, v) gather pair per queue: 4 pairs
            # generate concurrently, ~1024 descriptors each.
            for r in range(VRUNS):
                nc.gpsimd.dma_gather(
                    tu3[:, r * VBLK : (r + 1) * VBLK, :],
                    tbl[0:WIN, :],
                    t_idx[:, r * 2 * VCOLS : r * 2 * VCOLS + VCOLS],
                    VRUN,
                    VRUN,
                    D,
                    queue_num=r,
                    single_packet=False,
                )
            for r in range(VRUNS):
                c0 = r * 2 * VCOLS + VCOLS
                nc.gpsimd.dma_gather(
                    tv3[:, r * VBLK : (r + 1) * VBLK, :],
                    tbl[(1 + r) * WIN : (2 + r) * WIN, :],
                    t_idx[:, c0 : c0 + VCOLS],
                    VRUN,
                    VRUN,
                    D,
                    queue_num=r,
                    single_packet=False,
                )
                w = prod_pool.tile([P, VBLK * D], mybir.dt.float32, tag="w")
                nc.vector.tensor_mul(
                    out=w[:],
                    in0=tu[:, r * VBLK * D : (r + 1) * VBLK * D],
                    in1=tv[:, r * VBLK * D : (r + 1) * VBLK * D],
                )
                rs = t_res[:, r * VBLK : (r + 1) * VBLK]
                nc.vector.reduce_sum(
                    out=rs,
                    in_=w[:].rearrange("p (n d) -> p n d", d=D),
                    axis=mybir.AxisListType.X,
                )
                nc.scalar.activation(
                    out=rs,
                    in_=rs,
                    func=mybir.ActivationFunctionType.Sigmoid,
                    bias=t_bias[:],
                )
                nc.sync.dma_start(
                    out=out[:, r * VBLK : (r + 1) * VBLK], in_=rs
                )

    nc.compile()
    return nc


def _get_program():
    if "p" not in _programs:
        _programs["p"] = _build()
    return _programs["p"]


def _wrap16(ids: np.ndarray) -> np.ndarray:
    """Index list -> [128, n/16] int16 tile block (idx i at [i%16, i//16],
    replicated 8x down the partitions for the 8 Q7 cores)."""
    n = ids.shape[0]
    w = ids.reshape(n // 16, 16).T.astype(np.int16)  # [16, n/16]
    return np.tile(w, (8, 1))


def _prep(x: np.ndarray, user_table: np.ndarray, item_table: np.ndarray):
    """Sort/deal batch rows, build per-core idx tiles + table windows.

    Returns (in_maps, perm) where perm[k][i] is the batch row computed at
    position i of core k.
    """
    u_ids = x[:, 0].astype(np.int64)
    v_ids = x[:, 1].astype(np.int64)
    order = np.argsort(u_ids, kind="stable")
    in_maps = []
    perm = np.empty((N_CORES, BPC), dtype=np.int64)
    for k in range(N_CORES):
        sel = order[k * BPC : (k + 1) * BPC]
        sub = sel[np.argsort(v_ids[sel], kind="stable")]
        perm[k] = sub
        cu = u_ids[sub]
        cv = v_ids[sub]

        u_base = int(cu.min())
        if int(cu.max()) - u_base >= WIN:
            raise ValueError("user id span exceeds int16 gather window")

        tbl = np.zeros(((1 + VRUNS) * WIN, D), dtype=np.float32)
        take = min(WIN, user_table.shape[0] - u_base)
        tbl[:take] = user_table[u_base : u_base + take]

        idx_blocks = []
        for r in range(VRUNS):
            idx_blocks.append(_wrap16(cu[r * VRUN : (r + 1) * VRUN] - u_base))
            seg = cv[r * VRUN : (r + 1) * VRUN]
            v_base = int(seg[0])  # sorted ascending
            if int(seg[-1]) - v_base >= WIN:
                raise ValueError("item id span exceeds int16 gather window")
            idx_blocks.append(_wrap16(seg - v_base))
            take = min(WIN, item_table.shape[0] - v_base)
            tbl[(1 + r) * WIN : (1 + r) * WIN + take] = item_table[
                v_base : v_base + take
            ]

        in_maps.append(
            {
                "idx": np.ascontiguousarray(np.concatenate(idx_blocks, axis=1)),
                "tbl": tbl,
            }
        )
    return in_maps, perm


def _run(x, user_table, item_table, **run_kwargs):
    x = np.asarray(x)
    ut = np.asarray(user_table, dtype=np.float32)
    it = np.asarray(item_table, dtype=np.float32)
    assert x.shape == (B, 2), x.shape
    in_maps, perm = _prep(x, ut, it)
    nc = _get_program()
    res = run_bass_kernel_spmd(nc, in_maps, list(range(N_CORES)), **run_kwargs)
    out = np.empty(B, np.float32)
    for k in range(N_CORES):
        out[perm[k]] = res.results[k]["out"].T.ravel()
    return out, res


def kernel(x, user_table, item_table):
    out, _ = _run(x, user_table, item_table)
    return out
